# revision 2
# baseline (speedup 1.0000x reference)
"""GQA attention block (wq/wk/wv -> RoPE -> attention -> wo) on 8 TRN2 cores.

Sharding: tensor-parallel over heads. Core j owns kv-head j and q-heads
{j, j+8, j+16, j+24}. Each core computes a full [S, E] partial of the output
projection (contraction over its 256 head-dims of wo); partials are summed on
the host (bf16 partials, fp32 host accumulation).

v3 layout strategy (per core; matmul operands bf16, accumulation fp32):
  - Single 4-buf PSUM pool; every buf is [128,1024] f32 (2 banks). In the
    attention inner loop: 2 bufs rotate score pair-tiles (double-buffered,
    so the PE never waits on the exp pass), 1 buf holds BOTH AV accumulators
    (hh0/hh1 in separate banks of one tile), and 1 buf is spare for the
    drain (bt pair), Q-projection pair, and wo partials of the neighbouring
    pipeline stages.
  - Scores for both q-heads of a pack land in ONE [128,1024] pair-tile
    (hh0 cols 0:512, hh1 cols 512:1024); one ACT exp instruction per k-tile
    covers both heads (FD=1024), bias folds the attention mask.
  - Work is pipelined over 512-column q-blocks: Q-projection chunk -> RoPE
    chunk -> attention (both packs) -> output projection rows. K/V arrive
    early via 512-column key-chunk projection + per-chunk RoPE/transpose,
    and attention consumes K/V tiles at per-k-tile granularity, so the
    scores stream starts while later key chunks are still projecting.
  - Softmax denominator rides the AV matmul as a 65th column of ones in V.
  - All PSUM evictions go through the Vector engine; the Scalar engine runs
    only the exp stream (it is the phase-3 critical resource).
"""

import sys

sys.path.insert(0, "/opt/trn_rl_repo")

from contextlib import ExitStack

import ml_dtypes
import numpy as np

import concourse.bacc as bacc
import concourse.bass as bass
import concourse.tile as tile
from concourse import mybir
from concourse.bass_utils import run_bass_kernel_spmd

P = 128
S = 2048  # sequence length
E = 2048  # embed dim
D = 64    # head dim
EK = E // P   # 16 contraction tiles for projections
SK = S // P   # 16 key tiles for attention
NCORES = 8
QW = 512      # q-block width
NQB = S // QW  # 4 q-blocks
KC = 512      # key-chunk width for the KV projection
NKC = S // KC  # 4 key chunks
F32 = mybir.dt.float32
BF16 = mybir.dt.bfloat16
AF = mybir.ActivationFunctionType
BF16NP = ml_dtypes.bfloat16


def build_bass(repeat=1):
    nc = bacc.Bacc()
    xqT = nc.declare_dram_parameter("xqT", [E, S], BF16, isOutput=False)
    xkvT = nc.declare_dram_parameter("xkvT", [E, S], BF16, isOutput=False)
    wqT = nc.declare_dram_parameter("wqT", [E, 256], BF16, isOutput=False)
    wkvT = nc.declare_dram_parameter("wkvT", [E, P], BF16, isOutput=False)
    woT = nc.declare_dram_parameter("woT", [256, E], BF16, isOutput=False)
    rqc = nc.declare_dram_parameter("rqc", [P, S], BF16, isOutput=False)
    rqs = nc.declare_dram_parameter("rqs", [P, S], BF16, isOutput=False)
    rkc = nc.declare_dram_parameter("rkc", [D, S], BF16, isOutput=False)
    rks = nc.declare_dram_parameter("rks", [D, S], BF16, isOutput=False)
    mbias = nc.declare_dram_parameter("mbias", [P, SK], F32, isOutput=False)
    ident = nc.declare_dram_parameter("ident", [P, P], BF16, isOutput=False)
    outp = nc.declare_dram_parameter("out_partial", [S, E], BF16, isOutput=True)

    with ExitStack() as ctx:
        tc = ctx.enter_context(tile.TileContext(nc))
        persist = ctx.enter_context(tc.tile_pool(name="persist", bufs=1))

        qt0 = persist.tile([P, S], BF16, tag="qt0")
        qt1 = persist.tile([P, S], BF16, tag="qt1")
        qt = [qt0, qt1]
        ktdup = persist.tile([P, S], BF16, tag="ktdup")
        v_sb = persist.tile([P, SK, 65], BF16, tag="v_sb")  # V natural + ones col
        kv_sb = persist.tile([P, S], BF16, tag="kv_sb")  # KT(0:64)+VT(64:128)
        oA = persist.tile([P, S], BF16, tag="oA")
        oB = persist.tile([P, S], BF16, tag="oB")
        mb_sb = persist.tile([P, SK], F32, tag="mb_sb")
        id_sb = persist.tile([P, P], BF16, tag="id_sb")
        wq_sb = persist.tile([P, EK, 256], BF16, tag="wq_sb")
        wkv_sb = persist.tile([P, EK, P], BF16, tag="wkv_sb")
        wo_sb = persist.tile([P, 2, S], BF16, tag="wo_sb")
        rq_c = persist.tile([P, S], BF16, tag="rq_c")
        rq_s = persist.tile([P, S], BF16, tag="rq_s")
        rk_c = persist.tile([D, S], BF16, tag="rk_c")
        rk_s = persist.tile([D, S], BF16, tag="rk_s")
        ones_sb = persist.tile([1, P], BF16, tag="ones_sb")
        nc.vector.memset(ones_sb[:], 1.0)

        # weight DMAs in the order compute consumes them
        nc.scalar.dma_start(
            out=wkv_sb[:], in_=wkvT.ap().rearrange("(k p) c -> p k c", p=P)
        )
        nc.scalar.dma_start(out=rk_c[:], in_=rkc[:])
        nc.scalar.dma_start(out=rk_s[:], in_=rks[:])
        nc.scalar.dma_start(out=id_sb[:], in_=ident[:])
        nc.scalar.dma_start(out=mb_sb[:], in_=mbias[:])
        wq_r = wqT.ap().rearrange("(k p) c -> p k c", p=P)

        # PSUM pools (8 banks total): score pair-tiles double-buffer in their
        # own pool so the exp stream never blocks on boundary work; the AV
        # accumulator pair has a dedicated slot; everything else (Q-proj
        # halves, bt halves, wo chunks, KV-proj) shares two 1-bank bufs.
        spool = ctx.enter_context(tc.tile_pool(name="spool", bufs=2, space="PSUM"))
        uppool = ctx.enter_context(tc.tile_pool(name="uppool", bufs=1, space="PSUM"))
        aux = ctx.enter_context(tc.tile_pool(name="aux", bufs=2, space="PSUM"))
        xqpool = ctx.enter_context(tc.tile_pool(name="xqpool", bufs=2))
        xkpool = ctx.enter_context(tc.tile_pool(name="xkpool", bufs=2))
        epool = ctx.enter_context(tc.tile_pool(name="epool", bufs=4))
        upool = ctx.enter_context(tc.tile_pool(name="upool", bufs=2))
        rcpool = ctx.enter_context(tc.tile_pool(name="rcpool", bufs=2))
        otpool = ctx.enter_context(tc.tile_pool(name="otpool", bufs=2))
        swpool = ctx.enter_context(tc.tile_pool(name="swpool", bufs=2))
        skpool = ctx.enter_context(tc.tile_pool(name="skpool", bufs=2))
        ostage = ctx.enter_context(tc.tile_pool(name="ostage", bufs=3))

        def kv_chunk_items(rep, kc, xkt, wait_ms=0.0):
            """KV projection for key-chunk kc + RoPE-K + V transpose, as a
            list of deferred emission items (4 MM groups, finish, 2x Vt)."""
            cs = slice(kc * KC, (kc + 1) * KC)
            kvp = aux.tile([P, KC], F32, tag="ax", name=f"r{rep}_kvp{kc}")

            def mm_group(pc):
                def emit():
                    with tc.tile_wait_until(wait_ms + 0.0015 * pc, enable=wait_ms > 0):
                        for kt in range(pc * 4, (pc + 1) * 4):
                            nc.tensor.matmul(
                                kvp[:],
                                wkv_sb[:, kt, :],
                                xkt[:, kt, :],
                                start=(kt == 0),
                                stop=(kt == EK - 1),
                            )
                return emit

            def fin():
                with tc.tile_wait_until(wait_ms + 0.006, enable=wait_ms > 0):
                    _fin_body()

            def _fin_body():
                nc.vector.tensor_copy(kv_sb[:, cs], kvp[:])
                swk = skpool.tile([D, KC], BF16, tag="sk", name=f"r{rep}_swk{kc}")
                nc.gpsimd.dma_start(out=swk[0:32, :], in_=kv_sb[32:64, cs])
                nc.gpsimd.dma_start(out=swk[32:64, :], in_=kv_sb[0:32, cs])
                nc.vector.tensor_mul(ktdup[0:D, cs], kv_sb[0:D, cs], rk_c[:, cs])
                nc.vector.tensor_mul(swk[:], swk[:], rk_s[:, cs])
                nc.vector.tensor_add(ktdup[0:D, cs], ktdup[0:D, cs], swk[:])
                nc.gpsimd.dma_start(out=ktdup[D:P, cs], in_=ktdup[0:D, cs])

            def vt_pair(t0):
                def emit():
                    for t in (t0, t0 + 1):
                        kt = kc * (KC // P) + t
                        tp = aux.tile([P, D], BF16, tag="ax", name=f"r{rep}_vtp{kt}")
                        nc.tensor.transpose(
                            tp[:, :],
                            kv_sb[D:P, kt * P:(kt + 1) * P],
                            id_sb[D:P, D:P],
                        )
                        nc.vector.tensor_copy(v_sb[:, kt, 0:D], tp[:, :])
                return emit

            return [mm_group(0), mm_group(1), mm_group(2), mm_group(3),
                    fin, vt_pair(0), vt_pair(2)]

        def kv_chunk(rep, kc, xkt):
            for it in kv_chunk_items(rep, kc, xkt):
                it()

        def qproj_mm_list(rep, qb, xqt):
            """Q-projection matmuls for q-block qb as deferred emission items."""
            qps = [
                aux.tile([P, QW], F32, tag="ax", name=f"r{rep}_qp{qb}{p_}")
                for p_ in range(2)
            ]

            def mk(kt, p_):
                def emit():
                    nc.tensor.matmul(
                        qps[p_][:],
                        wq_sb[:, kt, p_ * P:(p_ + 1) * P],
                        xqt[:, kt, :],
                        start=(kt == 0),
                        stop=(kt == EK - 1),
                    )
                return emit

            items = [mk(kt, p_) for kt in range(EK) for p_ in range(2)]
            return qps, items

        def finish_q(rep, qb, qps):
            """Evict the Q-projection and apply RoPE for q-block qb."""
            qs = slice(qb * QW, (qb + 1) * QW)
            for p_ in range(2):
                nc.vector.tensor_copy(qt[p_][:, qs], qps[p_][:])
                sw = swpool.tile([P, QW], BF16, tag="sw", name=f"r{rep}_sw{qb}{p_}")
                for blk in range(4):
                    sb = blk ^ 1
                    q_eng = nc.gpsimd if blk < 2 else nc.sync
                    q_eng.dma_start(
                        out=sw[blk * 32:(blk + 1) * 32, :],
                        in_=qt[p_][sb * 32:(sb + 1) * 32, qs],
                    )
                nc.vector.tensor_mul(qt[p_][:, qs], qt[p_][:, qs], rq_c[:, qs])
                nc.vector.tensor_mul(sw[:], sw[:], rq_s[:, qs])
                nc.vector.tensor_add(qt[p_][:, qs], qt[p_][:, qs], sw[:])

        def xk_dma(rep, kc):
            xkt = xkpool.tile([P, EK, KC], BF16, tag="xk", name=f"r{rep}_xk{kc}")
            for pc in range(4):
                nc.sync.dma_start(
                    out=xkt[:, pc * 4:(pc + 1) * 4, :],
                    in_=xkvT[
                        pc * 4 * P:(pc + 1) * 4 * P, kc * KC:(kc + 1) * KC
                    ].rearrange("(k p) s -> p k s", p=P),
                )
            return xkt

        def xq_dma(rep, qb):
            xqt = xqpool.tile([P, EK, QW], BF16, tag="xq", name=f"r{rep}_xq{qb}")
            for pc in range(4):
                nc.sync.dma_start(
                    out=xqt[:, pc * 4:(pc + 1) * 4, :],
                    in_=xqT[
                        pc * 4 * P:(pc + 1) * 4 * P, qb * QW:(qb + 1) * QW
                    ].rearrange("(k p) s -> p k s", p=P),
                )
            return xqt

        def wo_item_list(rep, qb, alt_evict=False):
            """Output projection for q-block qb as deferred emission items."""
            items = []
            state = {}

            def mk(ms, ch):
                def emit():
                    if ch == 0:
                        state[ms] = ostage.tile(
                            [P, S], BF16, tag="ost", name=f"r{rep}_ost{ms}"
                        )
                    ost = state[ms]
                    e0 = ch * 512
                    if alt_evict and ch % 2 == 1:
                        wp = spool.tile([P, 512], F32, tag="sp", name=f"r{rep}_wp{ms}{ch}")
                    else:
                        wp = aux.tile([P, 512], F32, tag="ax", name=f"r{rep}_wp{ms}{ch}")
                    nc.tensor.matmul(
                        wp[:],
                        oA[:, ms * P:(ms + 1) * P],
                        wo_sb[:, 0, e0:e0 + 512],
                        start=True,
                        stop=False,
                    )
                    nc.tensor.matmul(
                        wp[:],
                        oB[:, ms * P:(ms + 1) * P],
                        wo_sb[:, 1, e0:e0 + 512],
                        start=False,
                        stop=True,
                    )
                    if alt_evict and ch % 2 == 1:
                        nc.scalar.copy(ost[:, e0:e0 + 512], wp[:])
                    else:
                        nc.vector.tensor_copy(ost[:, e0:e0 + 512], wp[:])
                    if ch == 3:
                        nc.sync.dma_start(
                            out=outp[ms * P:(ms + 1) * P, :], in_=ost[:]
                        )
                return emit

            for mi in range(QW // P):
                ms = qb * (QW // P) + mi
                for ch in range(4):
                    items.append(mk(ms, ch))
            return items

        def attention_block(rep, qb, pk, wo_items, qproj_items, finish_cb=None,
                            slot_items=None):
            """Attention for (qb, pk); drains deferred work into PE slack."""
            qs = slice(qb * QW, (qb + 1) * QW)
            ups = uppool.tile([P, 1024], F32, tag="up", name=f"r{rep}_u{qb}{pk}")
            for kt in range(SK):
                if slot_items and kt in slot_items:
                    wo_items.extend(slot_items.pop(kt))
                sps = spool.tile(
                    [P, 1024], F32, tag="sp", name=f"r{rep}_s{qb}{pk}{kt}"
                )
                for hh in range(2):
                    nc.tensor.matmul(
                        sps[:, hh * QW:(hh + 1) * QW],
                        ktdup[hh * D:(hh + 1) * D, kt * P:(kt + 1) * P],
                        qt[pk][hh * D:(hh + 1) * D, qs],
                        start=True,
                        stop=True,
                    )
                et = epool.tile([P, 1024], BF16, tag="e", name=f"r{rep}_e{qb}{pk}{kt}")
                nc.scalar.activation(
                    et[:],
                    sps[:],
                    AF.Exp,
                    bias=mb_sb[:, kt:kt + 1],
                    scale=0.125,
                )
                for hh in range(2):
                    nc.tensor.matmul(
                        ups[0:65, hh * QW:(hh + 1) * QW],
                        v_sb[:, kt, :],
                        et[:, hh * QW:(hh + 1) * QW],
                        start=(kt == 0),
                        stop=(kt == SK - 1),
                    )
                # fill PE slack with deferred wo / next-block Q-proj work
                for _ in range(2):
                    if wo_items:
                        wo_items.pop(0)()
                for _ in range(4):
                    if qproj_items:
                        qproj_items.pop(0)()
                if qproj_items is not None and not qproj_items and finish_cb:
                    finish_cb()
                    finish_cb = None
            return ups

        def drain_block(rep, qb, pk, ups):
            """Evict U, divide by the softmax denominators, place into packs."""
            qs = slice(qb * QW, (qb + 1) * QW)
            u_sb = upool.tile([65, 1024], F32, tag="u_sb", name=f"r{rep}_us{qb}{pk}")
            nc.vector.tensor_copy(u_sb[:], ups[0:65, :])
            rc = rcpool.tile([1, 1024], BF16, tag="rc", name=f"r{rep}_rc{qb}{pk}")
            with nc.allow_low_precision(
                reason="softmax denom recip at bf16 matches bf16 pipeline"
            ):
                nc.vector.reciprocal(rc[0:1, :], u_sb[D:D + 1, :])
            for hh in range(2):
                hs = slice(hh * QW, (hh + 1) * QW)
                bt = aux.tile([D, QW], F32, tag="ax", name=f"r{rep}_b{qb}{pk}{hh}")
                nc.tensor.matmul(
                    bt[:],
                    ones_sb[0:1, 0:D],
                    rc[0:1, hs],
                    start=True,
                    stop=True,
                )
                dest = oA if hh == 0 else oB
                if pk == 0:
                    nc.vector.tensor_mul(dest[0:D, qs], u_sb[0:D, hs], bt[:])
                else:
                    ot = otpool.tile([D, QW], BF16, tag="ot", name=f"r{rep}_ot{qb}{hh}")
                    nc.vector.tensor_mul(ot[:], u_sb[0:D, hs], bt[:])
                    nc.gpsimd.dma_start(out=dest[D:P, qs], in_=ot[:])

        for rep in range(repeat):
            nc.vector.memset(v_sb[:, :, D:65], 1.0)
            # PE warm-up: ~4us of tiny matmuls so the p-state ramp completes
            # before the first projection matmul issues
            wmt = aux.tile([D, D], F32, tag="ax", name=f"r{rep}_warm")
            for i in range(32):
                nc.tensor.matmul(
                    wmt[:], ones_sb[0:1, 0:D], ones_sb[0:1, 0:D],
                    start=True, stop=True,
                )

            # critical-path DMA order on the serial bus: xkv0 whole, then
            # wq/xq0 interleaved, then the remaining key chunks
            xkt0 = xk_dma(rep, 0)
            xqt0 = xqpool.tile([P, EK, QW], BF16, tag="xq", name=f"r{rep}_xq0")
            for pc in range(4):
                ks = slice(pc * 4 * P, (pc + 1) * 4 * P)
                kd = slice(pc * 4, (pc + 1) * 4)
                if rep == 0:
                    nc.sync.dma_start(out=wq_sb[:, kd, :], in_=wq_r[:, kd, :])
                nc.sync.dma_start(
                    out=xqt0[:, kd, :],
                    in_=xqT[ks, 0:QW].rearrange("(k p) s -> p k s", p=P),
                )
            if rep == 0:
                nc.scalar.dma_start(out=rq_c[:, 0:QW], in_=rqc[:, 0:QW])
                nc.scalar.dma_start(out=rq_s[:, 0:QW], in_=rqs[:, 0:QW])
            xkts = [None, xk_dma(rep, 1), xk_dma(rep, 2), xk_dma(rep, 3)]
            if rep == 0:
                nc.scalar.dma_start(out=rq_c[:, QW:S], in_=rqc[:, QW:S])
                nc.scalar.dma_start(out=rq_s[:, QW:S], in_=rqs[:, QW:S])
                nc.sync.dma_start(
                    out=wo_sb[:],
                    in_=woT.ap().rearrange("(k p) c -> p k c", p=P),
                )
            kv_chunk(rep, 0, xkt0)
            qps0, qitems0 = qproj_mm_list(rep, 0, xqt0)
            for it in qitems0:
                it()
            finish_q(rep, 0, qps0)
            kv_slots = {
                4 * (kc - 1): kv_chunk_items(
                    rep, kc, xkts[kc],
                    wait_ms=(0.0205 + 0.0058 * (kc - 1)) if rep == 0 else 0.0,
                )
                for kc in range(1, NKC)
            }

            # ======== main pipeline over q-blocks ========
            wo_items = []
            for qb in range(NQB):
                if qb + 1 < NQB:
                    xqt_next = xq_dma(rep, qb + 1)
                    qps_next, qproj_items = qproj_mm_list(rep, qb + 1, xqt_next)
                else:
                    qps_next, qproj_items = None, []
                fin = (
                    (lambda q=qb + 1, s=qps_next: finish_q(rep, q, s))
                    if qps_next is not None
                    else None
                )
                for pk in range(2):
                    ups = attention_block(
                        rep, qb, pk,
                        wo_items,
                        qproj_items if pk == 1 else [],
                        finish_cb=fin if pk == 1 else None,
                        slot_items=kv_slots if (qb == 0 and pk == 0) else None,
                    )
                    if pk == 1:
                        for it in qproj_items:
                            it()
                        qproj_items.clear()
                        if fin is not None and pk == 1 and qproj_items is not None:
                            pass
                    drain_block(rep, qb, pk, ups)
                wo_items.extend(wo_item_list(rep, qb, alt_evict=(qb == NQB - 1)))
            for it in wo_items:
                it()

    nc.compile()
    return nc


_PERM = np.concatenate([np.arange(0, D, 2), np.arange(1, D, 2)])


def _host_inputs(inputs):
    """Build the shared and per-core device input maps."""
    q = np.asarray(inputs["query_states"], np.float32)[0].T.astype(BF16NP)
    kv = np.asarray(inputs["key_value_states"], np.float32)[0].T.astype(BF16NP)
    wq = np.asarray(inputs["wq"], np.float32)
    wk = np.asarray(inputs["wk"], np.float32)
    wv = np.asarray(inputs["wv"], np.float32)
    wo = np.asarray(inputs["wo"], np.float32)
    cos_q = np.asarray(inputs["cos_q"], np.float32)
    sin_q = np.asarray(inputs["sin_q"], np.float32)
    cos_k = np.asarray(inputs["cos_k"], np.float32)
    sin_k = np.asarray(inputs["sin_k"], np.float32)
    mask = np.asarray(inputs["attention_mask"]).reshape(S)

    def rope_arrays(cos, sin, rows):
        ct = np.ascontiguousarray(cos.T)  # [32, S]
        st = np.ascontiguousarray(sin.T)
        rc = np.tile(ct, (rows // 32, 1))
        rs = np.tile(np.concatenate([-st, st], axis=0), (rows // 64, 1))
        return (
            np.ascontiguousarray(rc.astype(BF16NP)),
            np.ascontiguousarray(rs.astype(BF16NP)),
        )

    rq_c, rq_s = rope_arrays(cos_q, sin_q, P)
    rk_c, rk_s = rope_arrays(cos_k, sin_k, D)
    mb = np.where(mask, 0.0, -30000.0).astype(np.float32)
    mb = np.ascontiguousarray(mb.reshape(SK, P).T)  # [P, SK]
    ident = np.eye(P, dtype=BF16NP)

    shared = {
        "xqT": np.ascontiguousarray(q),
        "xkvT": np.ascontiguousarray(kv),
        "rqc": rq_c,
        "rqs": rq_s,
        "rkc": rk_c,
        "rks": rk_s,
        "mbias": mb,
        "ident": ident,
    }

    in_maps = []
    for j in range(NCORES):
        heads = [j, j + 8, j + 16, j + 24]
        wqTh = np.empty((E, 256), np.float32)
        for i, h in enumerate(heads):
            wqTh[:, i * D:(i + 1) * D] = wq[h * D + _PERM, :].T
        wk_p = wk[j * D + _PERM, :].T  # [E, 64]
        wv_p = wv[j * D:(j + 1) * D, :].T  # [E, 64]
        wkvTh = np.concatenate([wk_p, wv_p], axis=1)
        # O packs: oA rows = heads (0, 2), oB rows = heads (1, 3)
        woTh = np.empty((256, E), np.float32)
        for slot, h in enumerate([heads[0], heads[2], heads[1], heads[3]]):
            woTh[slot * D:(slot + 1) * D, :] = wo[:, h * D:(h + 1) * D].T
        in_maps.append(
            {
                **shared,
                "wqT": np.ascontiguousarray(wqTh.astype(BF16NP)),
                "wkvT": np.ascontiguousarray(wkvTh.astype(BF16NP)),
                "woT": np.ascontiguousarray(woTh.astype(BF16NP)),
            }
        )
    return in_maps


_NC_CACHE = {}


def _get_nc():
    if "nc" not in _NC_CACHE:
        _NC_CACHE["nc"] = build_bass()
    return _NC_CACHE["nc"]


def kernel(_trace=False, **inputs):
    nc = _get_nc()
    in_maps = _host_inputs(inputs)
    res = run_bass_kernel_spmd(
        nc, in_maps, core_ids=list(range(NCORES)), trace=_trace
    )
    out = np.zeros((S, E), np.float32)
    for r in res.results:
        out += r["out_partial"].astype(np.float32)
    if _trace:
        kernel.last_exec_time_ns = res.exec_time_ns
        kernel.last_results = res
    return out.reshape(1, S, E)
